# revision 35
# baseline (speedup 1.0000x reference)
"""AttnBlock (GroupNorm -> QKV 1x1 conv -> single-head attention over 4096
tokens -> proj -> residual) on 8 Trainium2 NeuronCores, batch-parallel
(one sample per core).

Design (vs the original DoubleRow-fp8 version):
 - attention matmuls are plain bf16 (no DoubleRow): HW probes showed the DR
   fp8 inner loop at ~2.87us/iter vs ~2.5us/iter plain, because DR disables
   fast-weight-load and pays heavy LDWEIGHTS per matmul
 - q/k/v/e all bf16 (also drops rel-err ~7e-3 -> ~1e-3)
 - 4 transposes batched into one PSUM bank + one strided copy out,
   copies alternate DVE/ACT; GN stats are chunked partial sums riding the
   transpose stream; group broadcast via one-hot matmul
 - QKV chunk emission is interleaved with the first q-chunk's attention
   steps (one chunk of lag) so the shared PSUM "sc" slots alternate between
   producer and consumer instead of draining all of QKV first
 - attention loop emits scores(s+1) before av(s) so PE is never queued
   behind the ACT exp of the current step; ACT does only exp in phase 3
 - activation-table sets preloaded via dummy ops (no mid-stream loads)
 - proj bias folded into a K=1 ones x bias_row matmul; residual add reads
   x_nat directly

Self-contained: hardcodes shapes b,h,w,c = 8,64,64,256 and builds/executes a
Bass/Tile kernel via run_bass_kernel_spmd.
"""

import sys

import numpy as np

if "/opt/trn_rl_repo" not in sys.path:
    sys.path.insert(0, "/opt/trn_rl_repo")

import concourse.bass as bass
import concourse.tile as tile
from concourse import bacc, mybir
from concourse.bass_utils import run_bass_kernel_spmd

F32 = mybir.dt.float32
BF16 = mybir.dt.bfloat16
FP8E4 = mybir.dt.float8e4  # e4m3 (TRN range +-240)
FP8E5 = mybir.dt.float8e5  # e5m2

B = 8
H = 64
W = 64
T = H * W          # 4096 tokens per sample
C = 256            # channels
P = 128            # partitions
CH = C // P        # 2 channel halves
TT = T // P        # 32 token tiles
QCS = 512          # q-chunk size (PSUM bank = 512 f32)
NQ = T // QCS      # 8 chunks
G = 32             # groups
GS = C // G        # 8 channels per group
EPS = 1e-6
N_GROUP = T * GS   # elements per group stat
SCALE = C ** -0.5  # softmax scale 1/16

AF = mybir.ActivationFunctionType
ALU = mybir.AluOpType
DR = mybir.MatmulPerfMode.DoubleRow
USE_DR = False  # DoubleRow fp8 vs plain bf16 (FWL, 2 el/cycle rhs) attention
E_BF16 = True   # q/k/v/e in bf16 instead of fp8
E_DT = BF16 if E_BF16 else FP8E5
QK_DT = BF16 if E_BF16 else FP8E4
V_DT = BF16 if E_BF16 else FP8E4
assert not (USE_DR and E_BF16), "DoubleRow needs fp8 operands"


def _group_consts():
    gsel = np.zeros((P, CH, G), np.float32)   # [p, h, g] one-hot: channel->group
    gbro = np.zeros((G, CH, P), np.float32)   # [g, h, p] one-hot: group->channel
    for h in range(CH):
        for p in range(P):
            g = (h * P + p) // GS
            gsel[p, h, g] = 1.0
            gbro[g, h, p] = 1.0
    return gsel, gbro


def _emit(tc, nc, xd, wd, bd, gsd, gbd, gseld, gbrod, identd, outd):
    ctxpools = []

    def pool(name, bufs, space="SBUF"):
        p = tc.alloc_tile_pool(name=name, bufs=bufs, space=space)
        ctxpools.append(p)
        return p

    const = pool("const", 1)
    stat = pool("stat", 1)
    work = pool("work", 2)
    epool = pool("epool", 6)
    # PSUM 8 banks: av0/av1/sps 3 + sc 2x2banks + small 1
    ps_acc = pool("ps_acc", 1, space="PSUM")
    ps_sc = pool("ps_sc", 2, space="PSUM")
    ps_sm = pool("ps_sm", 1, space="PSUM")

    x_view = xd[:, :].rearrange("(n p) c -> p n c", p=P)
    out_view = outd[:, :].rearrange("(n p) c -> p n c", p=P)

    # ---------------- x + identity first (phase-1 critical path) ----------------
    big = pool("big", 1)
    x_nat = big.tile([P, TT, C], F32)     # natural layout, 4 MB
    ident_sb = const.tile([P, P], F32)
    nc.sync.dma_start(out=ident_sb, in_=identd[:, :])
    for i in range(8):
        nc.sync.dma_start(
            out=x_nat[:, i * 4:(i + 1) * 4, :], in_=x_view[:, i * 4:(i + 1) * 4, :]
        )

    # ---------------- constants / weights (Activation HWDGE queue, so the
    # x load has the SP queue to itself) ----------------
    gsel_sb = const.tile([P, CH, G], F32)
    nc.scalar.dma_start(out=gsel_sb, in_=gseld[:, :, :])
    gbro_sb = const.tile([G, CH, P], F32)
    nc.scalar.dma_start(out=gbro_sb, in_=gbrod[:, :, :])
    ones_sb = const.tile([P, 2, P], E_DT)   # sps lhsT
    nc.vector.memset(ones_sb, 1.0)
    ones1_sb = const.tile([1, P], BF16)      # K=1 bias-matmul lhsT
    nc.vector.memset(ones1_sb, 1.0)

    wbf = {}
    for nm in ("q", "k", "v", "p"):
        w_sb = work.tile([P, CH, C], F32, tag="wload", bufs=4)
        nc.scalar.dma_start(out=w_sb, in_=wd[nm][:, :].rearrange("(h p) d -> p h d", p=P))
        wbf[nm] = const.tile([P, CH, C], BF16, name=f"wbf_{nm}")
        if nm == "q":
            nc.vector.tensor_scalar_mul(out=wbf[nm], in0=w_sb, scalar1=SCALE)
        else:
            nc.vector.tensor_copy(out=wbf[nm], in_=w_sb)

    bias_sb = {}
    for nm in ("q", "k"):
        b_sb = const.tile([P, CH], F32, name=f"bias_{nm}")
        nc.scalar.dma_start(out=b_sb, in_=bd[nm][:].rearrange("(h p) -> p h", p=P))
        bias_sb[nm] = b_sb
    bqs_sb = const.tile([P, CH], F32)
    nc.scalar.mul(out=bqs_sb, in_=bias_sb["q"], mul=SCALE)

    gns_sb = const.tile([P, CH], F32)
    nc.scalar.dma_start(out=gns_sb, in_=gsd[:].rearrange("(h p) -> p h", p=P))
    gnb_sb = const.tile([P, CH], F32)
    nc.scalar.dma_start(out=gnb_sb, in_=gbd[:].rearrange("(h p) -> p h", p=P))

    # bv replicated across partitions (v-bias DVE add); bp as a bf16 row
    # (folded into proj via a K=1 matmul)
    bv_rep = const.tile([P, C], F32)
    bcast = bass.AP(tensor=bd["v"], offset=0, ap=[[0, P], [1, C]])
    nc.gpsimd.dma_start(out=bv_rep, in_=bcast)
    bp_row_f = const.tile([1, C], F32)
    nc.scalar.dma_start(out=bp_row_f, in_=bass.AP(tensor=bd["p"], offset=0, ap=[[0, 1], [1, C]]))
    bp_row = const.tile([1, C], BF16)
    nc.vector.tensor_copy(out=bp_row, in_=bp_row_f)

    # ---------------- persistent big tensors ----------------
    xT = big.tile([P, CH, T], BF16)       # x^T bf16, 2 MB
    hT = big.tile([P, CH, T], BF16)       # groupnormed, bf16, 2 MB
    qT = big.tile([P, CH, T], QK_DT)
    kT = big.tile([P, CH, T], QK_DT)
    v_sb = big.tile([P, TT, C], V_DT)

    # Preload the sqrt table set during the initial DMA wait (covers Square/
    # Identity/Sqrt for phases 1-2); the exp set is preloaded after the GN
    # sqrt below so the attention stream never pays a mid-flow table load.
    dummy = stat.tile([1, 1], F32)
    nc.vector.memset(dummy, 1.0)
    dsink = stat.tile([1, 1], F32)
    nc.scalar.activation(out=dsink, in_=dummy, func=AF.Sqrt)

    # ---------------- phase 1: transposes (PE) + batched copies (DVE/ACT)
    # + chunked GN stats riding the transpose stream ----------------
    stp = stat.tile([P, CH, 8], F32)   # per-chunk partial sums
    sqp = stat.tile([P, CH, 8], F32)   # per-chunk partial sumsq
    for c in range(8):
        for g in range(2):  # two tile-pairs per chunk
            n = 4 * c + 2 * g
            tp = ps_sc.tile([P, 4, P], F32, tag="sc", name="tp")
            for i in range(4):
                nn, h = n + i // 2, i % 2
                nc.tensor.transpose(
                    tp[:, i, :], x_nat[:, nn, h * P:(h + 1) * P], ident_sb
                )
            out_ap = xT[:, :, n * P:(n + 2) * P].rearrange(
                "p h (n2 t) -> p n2 h t", n2=2
            )
            if g == 0:
                nc.vector.tensor_copy(out=out_ap, in_=tp)
            else:
                nc.scalar.copy(out=out_ap, in_=tp)
        sl = slice(c * QCS, (c + 1) * QCS)
        for h in range(CH):
            nc.vector.reduce_sum(
                out=stp[:, h, c:c + 1], in_=xT[:, h, sl], axis=mybir.AxisListType.X
            )
            # Square writes into hT as scratch (overwritten later by the affine)
            nc.scalar.activation(
                out=hT[:, h, sl], in_=xT[:, h, sl], func=AF.Square,
                accum_out=sqp[:, h, c:c + 1],
            )

    st4 = stat.tile([P, 4], F32)  # [sum_h0, sumsq_h0, sum_h1, sumsq_h1]
    for h in range(CH):
        nc.vector.reduce_sum(
            out=st4[:, 2 * h:2 * h + 1], in_=stp[:, h, :], axis=mybir.AxisListType.X
        )
        nc.vector.reduce_sum(
            out=st4[:, 2 * h + 1:2 * h + 2], in_=sqp[:, h, :], axis=mybir.AxisListType.X
        )

    gps = ps_sm.tile([G, 2], F32, tag="small")
    nc.tensor.matmul(gps, lhsT=gsel_sb[:, 0, :], rhs=st4[:, 0:2], start=True, stop=False)
    nc.tensor.matmul(gps, lhsT=gsel_sb[:, 1, :], rhs=st4[:, 2:4], start=False, stop=True)

    # gmr cols: 0 mean, 1 rstd
    gstat = stat.tile([G, 4], F32)
    nc.vector.tensor_scalar_mul(out=gstat[:, 0:2], in0=gps, scalar1=1.0 / N_GROUP)
    nc.vector.tensor_mul(out=gstat[:, 2:3], in0=gstat[:, 0:1], in1=gstat[:, 0:1])
    nc.vector.tensor_sub(out=gstat[:, 2:3], in0=gstat[:, 1:2], in1=gstat[:, 2:3])
    nc.vector.tensor_scalar_add(out=gstat[:, 2:3], in0=gstat[:, 2:3], scalar1=EPS)
    nc.scalar.activation(out=gstat[:, 2:3], in_=gstat[:, 2:3], func=AF.Sqrt)
    nc.vector.reciprocal(out=gstat[:, 2:3], in_=gstat[:, 2:3])
    # preload the exp table set now, before the attention stream needs it
    nc.scalar.activation(out=dsink, in_=dummy, func=AF.Exp)
    gmr = stat.tile([G, 2], F32)
    nc.vector.tensor_copy(out=gmr[:, 0:1], in_=gstat[:, 0:1])
    nc.vector.tensor_copy(out=gmr[:, 1:2], in_=gstat[:, 2:3])

    mr_sb = stat.tile([P, CH, 2], F32)  # per-channel [mean, rstd]
    for h in range(CH):
        mbc = ps_sm.tile([P, 2], F32, tag="small", name="mbc")
        nc.tensor.matmul(mbc, lhsT=gbro_sb[:, h, :], rhs=gmr, start=True, stop=True)
        nc.vector.tensor_copy(out=mr_sb[:, h, :], in_=mbc)

    m_sb = stat.tile([P, CH], F32)
    a_sb = stat.tile([P, CH], F32)
    nc.vector.tensor_mul(out=m_sb, in0=mr_sb[:, :, 1], in1=gns_sb)
    nc.vector.tensor_mul(out=a_sb, in0=mr_sb[:, :, 0], in1=m_sb)
    nc.vector.tensor_sub(out=a_sb, in0=gnb_sb, in1=a_sb)

    # ---------------- phases 2+3 interleaved ----------------
    # QKV chunk ck only unlocks key tiles < 4(ck+1); qc0's attention step s
    # needs key tiles 2s,2s+1, so after qkv(ck) we can run s = 2ck, 2ck+1.
    # This keeps the two PSUM "sc" slots alternating between producer and
    # consumer instead of draining all of QKV before attention starts.
    NS = TT // 2  # 16 double-key-tile steps per q-chunk

    def emit_affine_qkv(ck):
        sl = slice(ck * QCS, (ck + 1) * QCS)
        # hT = xT * m + a  (bf16 4x DVE)
        for h in range(CH):
            nc.vector.tensor_scalar(
                out=hT[:, h, sl], in0=xT[:, h, sl],
                scalar1=m_sb[:, h:h + 1], scalar2=a_sb[:, h:h + 1],
                op0=ALU.mult, op1=ALU.add,
            )
        for nm, dst in (("q", qT), ("k", kT)):
            ps = ps_sc.tile([P, CH, QCS], F32, tag="sc", name="psqk")
            for dh in range(CH):
                nc.tensor.matmul(
                    ps[:, dh, :], lhsT=wbf[nm][:, 0, dh * P:(dh + 1) * P],
                    rhs=hT[:, 0, sl], start=True, stop=False,
                )
                nc.tensor.matmul(
                    ps[:, dh, :], lhsT=wbf[nm][:, 1, dh * P:(dh + 1) * P],
                    rhs=hT[:, 1, sl], start=False, stop=True,
                )
                if nm == "q":  # q copies on DVE, k copies on ACT
                    nc.vector.tensor_scalar_add(
                        out=dst[:, dh, sl], in0=ps[:, dh, :],
                        scalar1=bqs_sb[:, dh:dh + 1],
                    )
                else:
                    nc.scalar.activation(
                        out=dst[:, dh, sl], in_=ps[:, dh, :], func=AF.Identity,
                        bias=bias_sb["k"][:, dh:dh + 1], scale=1.0,
                    )
        psv = ps_sc.tile([P, 4, C], F32, tag="sc", name="psv")
        for i, n in enumerate(range(4 * ck, 4 * ck + 4)):
            nc.tensor.matmul(
                psv[:, i, :], lhsT=hT[:, 0, n * P:(n + 1) * P], rhs=wbf["v"][:, 0, :],
                start=True, stop=False,
            )
            nc.tensor.matmul(
                psv[:, i, :], lhsT=hT[:, 1, n * P:(n + 1) * P], rhs=wbf["v"][:, 1, :],
                start=False, stop=True,
            )
            nc.vector.tensor_add(out=v_sb[:, n, :], in0=psv[:, i, :], in1=bv_rep)

    def attn_qc(qc):
        qsl = slice(qc * QCS, (qc + 1) * QCS)
        st = {
            "av0": ps_acc.tile([P, QCS], F32, tag="av0", name="av0"),
            "av1": ps_acc.tile([P, QCS], F32, tag="av1", name="av1"),
            "sps": ps_acc.tile([P, QCS], F32, tag="sps", name="sps"),
            "e": [None] * NS,
            "qsl": qsl,
            "qc": qc,
        }
        return st

    def emit_sc(st, s):
        e_pair = epool.tile([P, 2, QCS], E_DT, tag="e", name="e_pair")
        scp = ps_sc.tile([P, 2, QCS], F32, tag="sc", name="scp")
        for j in range(2):
            kt = 2 * s + j
            if USE_DR:
                nc.tensor.matmul(
                    scp[:, j, :], lhsT=kT[:, :, kt * P:(kt + 1) * P],
                    rhs=qT[:, :, st["qsl"]], start=True, stop=True, perf_mode=DR,
                )
            else:
                nc.tensor.matmul(
                    scp[:, j, :], lhsT=kT[:, 0, kt * P:(kt + 1) * P],
                    rhs=qT[:, 0, st["qsl"]], start=True, stop=False,
                )
                nc.tensor.matmul(
                    scp[:, j, :], lhsT=kT[:, 1, kt * P:(kt + 1) * P],
                    rhs=qT[:, 1, st["qsl"]], start=False, stop=True,
                )
        nc.scalar.activation(out=e_pair, in_=scp, func=AF.Exp)
        st["e"][s] = e_pair

    def emit_av(st, s):
        e_pair = st["e"][s]
        first = s == 0
        last = s == NS - 1
        if USE_DR:
            nc.tensor.matmul(
                st["av0"], lhsT=v_sb[:, 2 * s:2 * s + 2, 0:P], rhs=e_pair,
                start=first, stop=last, perf_mode=DR, skip_group_check=True,
            )
            nc.tensor.matmul(
                st["av1"], lhsT=v_sb[:, 2 * s:2 * s + 2, P:C], rhs=e_pair,
                start=first, stop=last, perf_mode=DR, skip_group_check=True,
            )
            nc.tensor.matmul(
                st["sps"], lhsT=ones_sb, rhs=e_pair,
                start=first, stop=last, perf_mode=DR, skip_group_check=True,
            )
        else:
            for j in range(2):
                fi = first and j == 0
                la = last and j == 1
                nc.tensor.matmul(
                    st["av0"], lhsT=v_sb[:, 2 * s + j, 0:P], rhs=e_pair[:, j, :],
                    start=fi, stop=la, skip_group_check=True,
                )
                nc.tensor.matmul(
                    st["av1"], lhsT=v_sb[:, 2 * s + j, P:C], rhs=e_pair[:, j, :],
                    start=fi, stop=la, skip_group_check=True,
                )
                nc.tensor.matmul(
                    st["sps"], lhsT=ones_sb[:, j, :], rhs=e_pair[:, j, :],
                    start=fi, stop=la, skip_group_check=True,
                )

    def emit_tail(st):
        qc = st["qc"]
        r = work.tile([P, QCS], F32, tag="r", name="r")
        nc.vector.reciprocal(out=r, in_=st["sps"])
        ao = work.tile([P, CH, QCS], BF16, tag="ao", name="ao")
        nc.vector.tensor_mul(out=ao[:, 0, :], in0=st["av0"], in1=r)
        nc.vector.tensor_mul(out=ao[:, 1, :], in0=st["av1"], in1=r)
        for g in range(2):  # two pairs of token tiles
            po = ps_sm.tile([P, 2, C], F32, tag="small", name="po")
            for tt in range(2):
                off = (2 * g + tt) * P
                nc.tensor.matmul(
                    po[:, tt, :], lhsT=ao[:, 0, off:off + P], rhs=wbf["p"][:, 0, :],
                    start=True, stop=False,
                )
                nc.tensor.matmul(
                    po[:, tt, :], lhsT=ao[:, 1, off:off + P], rhs=wbf["p"][:, 1, :],
                    start=False, stop=False,
                )
                nc.tensor.matmul(  # += bp (K=1 ones x bp_row)
                    po[:, tt, :], lhsT=ones1_sb, rhs=bp_row,
                    start=False, stop=True,
                )
            n = qc * 4 + 2 * g
            o_sb = work.tile([P, 2, C], F32, tag="o", name="o_sb")
            nc.vector.tensor_add(out=o_sb, in0=po, in1=x_nat[:, n:n + 2, :])
            nc.sync.dma_start(out=out_view[:, n:n + 2, :], in_=o_sb)

    # qc0 interleaved with QKV production, one chunk behind so attention
    # never waits on the key tiles produced in the same chunk
    st0 = attn_qc(0)
    for ck in range(NQ):
        emit_affine_qkv(ck)
        if ck >= 1:
            for s in (2 * (ck - 1), 2 * (ck - 1) + 1):
                emit_sc(st0, s)
                if s >= 1:
                    emit_av(st0, s - 1)
    for s in (2 * (NQ - 1), 2 * (NQ - 1) + 1):
        emit_sc(st0, s)
        emit_av(st0, s - 1)
    emit_av(st0, NS - 1)
    emit_tail(st0)

    # remaining q-chunks: standard pipelined loop
    for qc in range(1, NQ):
        st = attn_qc(qc)
        emit_sc(st, 0)
        for s in range(1, NS):
            emit_sc(st, s)
            emit_av(st, s - 1)
        emit_av(st, NS - 1)
        emit_tail(st)

    for p in reversed(ctxpools):
        p.release()


def build_nc():
    nc = bacc.Bacc()
    xd = nc.dram_tensor("x", [T, C], F32, kind="ExternalInput")
    wd, bd = {}, {}
    for nm in ("q", "k", "v", "p"):
        wd[nm] = nc.dram_tensor(f"w{nm}", [C, C], F32, kind="ExternalInput")
        bd[nm] = nc.dram_tensor(f"b{nm}", [C], F32, kind="ExternalInput")
    gsd = nc.dram_tensor("gn_scale", [C], F32, kind="ExternalInput")
    gbd = nc.dram_tensor("gn_bias", [C], F32, kind="ExternalInput")
    outd = nc.dram_tensor("out", [T, C], F32, kind="ExternalOutput")

    gsel_np, gbro_np = _group_consts()
    gseld = nc.inline_tensor(gsel_np, "gsel")
    gbrod = nc.inline_tensor(gbro_np, "gbro")
    identd = nc.inline_tensor(np.eye(P, dtype=np.float32), "ident")

    with tile.TileContext(nc) as tc:
        _emit(tc, nc, xd, wd, bd, gsd, gbd, gseld, gbrod, identd, outd)
    nc.compile()
    return nc


_CACHE = {}


def kernel(**inputs):
    x = np.asarray(inputs["x"], np.float32)
    assert x.shape == (B, H, W, C), x.shape
    if "nc" not in _CACHE:
        _CACHE["nc"] = build_nc()
    nc = _CACHE["nc"]

    shared = {}
    for nm in ("q", "k", "v", "p"):
        shared[f"w{nm}"] = np.ascontiguousarray(np.asarray(inputs[f"w{nm}"], np.float32))
        shared[f"b{nm}"] = np.ascontiguousarray(np.asarray(inputs[f"b{nm}"], np.float32))
    shared["gn_scale"] = np.ascontiguousarray(np.asarray(inputs["gn_scale"], np.float32))
    shared["gn_bias"] = np.ascontiguousarray(np.asarray(inputs["gn_bias"], np.float32))

    in_maps = []
    for i in range(B):
        m = dict(shared)
        m["x"] = np.ascontiguousarray(x[i].reshape(T, C))
        in_maps.append(m)

    res = run_bass_kernel_spmd(nc, in_maps, core_ids=list(range(B)))
    _CACHE["last_exec_time_ns"] = res.exec_time_ns
    out = np.stack([res.results[i]["out"].reshape(H, W, C) for i in range(B)], axis=0)
    return out
